# revision 1
# baseline (speedup 1.0000x reference)
"""Trainium2 Bass kernel for sparse (shared-prefix) GQA decode attention.

Full-input contract: kernel(**inputs) takes the unsharded tensors from
setup_inputs() and returns the full [16, 1, 4096] float32 output.

Sharding: tensor-parallel over heads across 8 NeuronCores. Core m owns
query heads 4m..4m+3 and kv head m (GQA group m), i.e. wq columns
[512m, 512m+512), wk/wv columns [128m, 128m+128), wo rows [512m, 512m+512),
and head m of the kv caches. Each core computes a partial output
y_m = attn_m @ wo_m; the host sums the 8 partials (the "all-reduce").

Device-side layout: scores are kept transposed, sT[j, r] with r = 4b+h on
the free dim, so every engine op starts at partition 0 (the hardware only
allows aligned partition bases) and the probabilities come out already in
the orientation the PV matmul needs.

Problem constants (hardcoded per the harness contract): bsz=16, seqlen=1,
dim=4096, n_heads=32, n_kv=8, hd=128, start_pos=2048,
shared_prefix_length=512 -> rsp=1536, L=2049.
"""

import math
import os
import sys
import types

import numpy as np

# ----------------------------------------------------------------------------
# environment patches (self-contained; no /root/problem reads)
# ----------------------------------------------------------------------------


def _patch_tile_drain():
    """The stock TileContext._drain_and_barrier puts one sem-wait per live
    semaphore on a single Drain instruction; the walrus build in this image
    only accepts a single sync wait per instruction ("Too many sync wait
    commands"). Re-emit the waits as individual EventSemaphore instructions
    on the same sequencer instead."""
    import concourse.tile as tile
    from concourse.vector_clock import ScopedClock

    if getattr(tile.TileContext, "_drain_patched", False):
        return

    def _drain_and_barrier(self, tick_clock, wait_clock):
        nc = self.nc
        drain_inst = nc.sync.drain()
        wait_clock.add_sem_waits(
            drain_inst.ins, ScopedClock({None: tick_clock.global_clock})
        )
        waits = list(drain_inst.ins.sync_info.on_wait)
        if len(waits) > 1:
            by_name = {h.name: h for h in self.sems.allocated().values()}
            try:
                drain_inst.ins.sync_info = None
            except Exception:
                pass
            for w in waits:
                h = by_name.get(w.ant_name)
                assert h is not None, f"no handle for sem {w.ant_name}"
                nc.sync.wait_ge(h, w.wait_value)

        # No barrier / explicit sem clears: every instruction transitively
        # precedes the SP wait chain above, and the NRT postamble already
        # resets all semaphores. Only do the python-side bookkeeping.
        assert self.sems is not None
        popped = nc._tile_sem_poison_stack.pop()
        assert popped is self._sem_poison
        nums = [h.num for h in self.sems.allocated().values()]
        nc._state.prepend_free_semaphores(nums)
        for ps in nc._tile_sem_poison_stack:
            ps.update(nums)

    tile.TileContext._drain_and_barrier = _drain_and_barrier
    tile.TileContext._drain_patched = True


def _install_ntff_hook():
    """Optional: register the axon NTFF profile hook (missing from the
    trimmed antenv package) so trace=True works for profiling, and stub the
    S3 artifact upload (zero-egress container)."""
    try:
        if "antenv.axon_hooks" not in sys.modules:
            mod = types.ModuleType("antenv.axon_hooks")
            mod._hook = None
            mod.set_axon_ntff_profile_hook = lambda h: setattr(mod, "_hook", h)
            mod.get_axon_ntff_profile_hook = lambda: mod._hook
            sys.modules["antenv.axon_hooks"] = mod
            import antenv

            antenv.axon_hooks = mod
            from trn_agent_boot.trn_boot import _ntff_profile_via_ctypes

            mod.set_axon_ntff_profile_hook(
                _ntff_profile_via_ctypes("/opt/axon/libaxon_pjrt.so")
            )
        import concourse.bass_utils as bu

        bu.upload_artifacts = lambda tmpdir: tmpdir
    except Exception:
        pass




def _legalize_multiwait(nc, max_waits=1):
    """This walrus build accepts at most one sync wait per instruction.
    Hoist excess waits into standalone single-wait EventSemaphore
    instructions inserted immediately before, on the same engine."""
    import bass_rust

    uid = 0
    for f in nc.m.functions:
        for bb in f.blocks:
            insts = list(bb.instructions)
            out = []
            changed = False
            for ins in insts:
                si = ins.sync_info
                if si is not None:
                    waits = list(si.on_wait)
                    if len(waits) > max_waits:
                        for w in waits[:-max_waits]:
                            ev = bass_rust.InstEventSemaphore(
                                name=f"{ins.name}_xw{uid}"
                            )
                            uid += 1
                            ev.engine = ins.engine
                            ev.sync_info = bass_rust.SyncInfo(
                                on_wait=[w], on_update=[]
                            )
                            out.append(ev)
                        ins.sync_info = bass_rust.SyncInfo(
                            on_wait=waits[-max_waits:],
                            on_update=list(si.on_update),
                        )
                        changed = True
                out.append(ins)
            if changed:
                bb.instructions = out


# ----------------------------------------------------------------------------
# constants
# ----------------------------------------------------------------------------

N_CORES = 8
B = 16            # batch
DIM = 4096
N_HEADS = 32
N_KV = 8
HD = 128
NH = N_HEADS // N_CORES      # 4 local q heads
R = B * NH                   # 64 (b,h) rows, r = 4*b + h
SOFTMAX_SCALE = 1.0 / math.sqrt(HD)
NEG_BIG = -1.0e30

# stream dtype for weights / kv-cache / matmul operands. "bfloat16" halves the
# HBM traffic (memory-bound kernel); softmax stays fp32 and all matmuls
# accumulate in fp32 PSUM.
STREAM_DTYPE = os.environ.get("KERNEL_STREAM_DTYPE", "bfloat16")
# use the fp32r (full-rate) matmul mode when streaming fp32
F32R = os.environ.get("KERNEL_F32R", "1") == "1"


# ----------------------------------------------------------------------------
# device kernel
# ----------------------------------------------------------------------------


def _build_nc(spl, rsp, dt_name):
    import concourse.bass as bass
    import concourse.tile as tile
    from concourse.tile import add_dep_helper
    from concourse import mybir
    from concourse.masks import make_identity
    from concourse.mybir import ActivationFunctionType as AF

    DT = getattr(mybir.dt, dt_name)
    f32 = mybir.dt.float32
    assert spl % 128 == 0 and rsp % 512 == 0
    NPAIR = B // 2              # kv batches are DMA'd in pairs
    SH_CH = spl // 128          # shared j-chunks (4)
    BCH = rsp // 128            # per-batch j-chunks (12)
    NCH = SH_CH + BCH + 1       # total chunks incl. new-token chunk (17)

    def mm(ap):
        # optionally reinterpret fp32 operands as fp32r for full-rate matmul
        if dt_name == "float32" and F32R:
            return ap.bitcast(mybir.dt.float32r)
        return ap

    nc = bass.Bass(
        "TRN2", target_bir_lowering=False, debug=False, num_devices=N_CORES
    )

    def din(name, shape, dt=DT):
        return nc.dram_tensor(name, shape, dt, kind="ExternalInput").ap()

    xv_scr = nc.dram_tensor("xv_scr", [B, HD], DT, kind="Internal").ap()
    rinv_scr = nc.dram_tensor("rinv_scr", [1, R], f32, kind="Internal").ap()
    cpack_d = din("cpack", [128, 32 * B + 2 * spl])
    wq_d = din("wq", [2, 128, 16 * 512])
    wkv_d = din("wkv", [128, 32 * 256])
    wo_d = din("wo", [4, 128, 2 * 4 * 512])
    kT_d = din("kT", [NPAIR // 2, 128, 4 * rsp])
    v_d = din("v", [NPAIR // 2, 128, 4 * rsp])
    rpack_d = din("rpack", [B, 2 * NH * 64], f32)
    y_d = nc.dram_tensor("y", [B, DIM], f32, kind="ExternalOutput").ap()

    with tile.TileContext(nc) as tc:
        with tc.tile_pool(name="const", bufs=1) as const, \
             tc.tile_pool(name="wpool", bufs=2) as wpool, \
             tc.tile_pool(name="kpool", bufs=3) as kpool, \
             tc.tile_pool(name="vpool", bufs=4) as vpool, \
             tc.tile_pool(name="wopool", bufs=4) as wopool, \
             tc.tile_pool(name="tmp", bufs=4) as tmp:

            # ---------------- resident tiles ----------------
            id_sb = const.tile([64, 64], DT)
            make_identity(nc, id_sb)
            ones_sb = const.tile([128, 1], DT)
            nc.vector.memset(ones_sb, 1.0)
            ones1p = const.tile([1, 128], DT)
            nc.vector.memset(ones1p, 1.0)
            cpack_sb = const.tile([128, 32 * B + 2 * spl], DT)
            nc.sync.dma_start(out=cpack_sb, in_=cpack_d)
            xT_sb = cpack_sb[:, : 32 * B]
            shkT_sb = cpack_sb[:, 32 * B : 32 * B + spl]
            shv_sb = cpack_sb[:, 32 * B + spl :]
            rpack_sb = const.tile([B, 2 * NH * 64], f32)
            nc.sync.dma_start(out=rpack_sb, in_=rpack_d)
            crep_sb = rpack_sb[:, : NH * 64]
            srep_sb = rpack_sb[:, NH * 64 :]

            qT_sb = const.tile([128, R], DT)        # cols r = 4b+h
            xkT_sb = const.tile([128, B], DT)
            xv_sb = const.tile([B, HD], DT)
            xv1p = const.tile([1, B * HD], DT)      # xv rows on one partition
            sT_sb = const.tile([128, NCH, R], f32)  # transposed scores
            pT_sb = const.tile([128, NCH, R], DT)   # transposed probabilities
            sum1_sb = const.tile([1, R], f32)
            rinv1_sb = const.tile([1, R], f32)
            rinv_bc = const.tile([128, R], DT)      # rinv broadcast, cols (h,b)
            attnT_sb = const.tile([128, R], DT)     # cols (h,b) = 16h + b
            attnTsh_sb = const.tile([128, R], DT)   # shared-prefix part
            pvsh_sb = const.tile([R, HD], DT)
            attnTn_sb = const.tile([128, R], DT)
            y_sb = const.tile([B, DIM], f32)

            # ---------------- phase A: projections + rope ----------------
            with tc.tile_pool(name="psA", bufs=1, space="PSUM") as psA, \
                 tc.tile_pool(name="ptrA", bufs=2, space="PSUM") as ptrA:
                xq_ps = psA.tile([B, NH * HD], f32)
                for g in range(2):
                    wt = wpool.tile([128, 16 * 512], DT, tag="wq", name="wt")
                    nc.sync.dma_start(out=wt, in_=wq_d[g])
                    for c in range(16):
                        k = 16 * g + c
                        nc.tensor.matmul(
                            xq_ps,
                            mm(xT_sb[:, B * k : B * (k + 1)]),
                            mm(wt[:, 512 * c : 512 * (c + 1)]),
                            start=(k == 0),
                            stop=(k == 31),
                        )
                xkv_ps = psA.tile([B, 2 * HD], f32)
                wkv_sb = const.tile([128, 32 * 256], DT)
                nc.sync.dma_start(out=wkv_sb, in_=wkv_d)
                for k in range(32):
                    nc.tensor.matmul(
                        xkv_ps,
                        mm(xT_sb[:, B * k : B * (k + 1)]),
                        mm(wkv_sb[:, 256 * k : 256 * (k + 1)]),
                        start=(k == 0),
                        stop=(k == 31),
                    )

                # rope: pairs (even, odd) along hd; cos/sin repeated per head
                def rope(dst, src_ps, width):
                    e = src_ps.rearrange("p (n two) -> p n two", two=2)[:, :, 0]
                    o = src_ps.rearrange("p (n two) -> p n two", two=2)[:, :, 1]
                    de = dst.rearrange("p (n two) -> p n two", two=2)[:, :, 0]
                    do = dst.rearrange("p (n two) -> p n two", two=2)[:, :, 1]
                    c_ap = crep_sb[:, :width]
                    s_ap = srep_sb[:, :width]
                    t1 = tmp.tile([B, NH * 64], f32, tag="t1", name="t1")[:, :width]
                    t2 = tmp.tile([B, NH * 64], f32, tag="t2", name="t2")[:, :width]
                    nc.vector.tensor_mul(t1, e, c_ap)
                    nc.vector.tensor_mul(t2, o, s_ap)
                    nc.vector.tensor_sub(de, t1, t2)
                    t3 = tmp.tile([B, NH * 64], f32, tag="t1", name="t3")[:, :width]
                    t4 = tmp.tile([B, NH * 64], f32, tag="t2", name="t4")[:, :width]
                    nc.vector.tensor_mul(t3, e, s_ap)
                    nc.vector.tensor_mul(t4, o, c_ap)
                    nc.vector.tensor_add(do, t3, t4)

                xq_r = const.tile([B, NH * HD], DT)
                rope(xq_r, xq_ps, NH * 64)
                xk_r = const.tile([B, HD], DT)
                rope(xk_r, xkv_ps[:, :HD], 64)
                nc.vector.tensor_copy(xv_sb, xkv_ps[:, HD:])

                # qT (cols r = 4b+h) via per-head PE transposes
                for h in range(NH):
                    tp = ptrA.tile([128, B], DT, tag="tq", name="tp")
                    nc.tensor.transpose(
                        tp, xq_r[:, HD * h : HD * (h + 1)], id_sb[:B, :B]
                    )
                    out_ap = qT_sb.rearrange("p (b h) -> p b h", h=NH)[:, :, h]
                    nc.vector.tensor_copy(out_ap, tp)
                tpk = ptrA.tile([128, B], DT, tag="tq", name="tpk")
                nc.tensor.transpose(tpk, xk_r, id_sb[:B, :B])
                nc.vector.tensor_copy(xkT_sb, tpk)

                # gather xv rows onto partition 0 (partition move -> via DRAM,
                # on the SWDGE queue so the SP HWDGE stream is not blocked)
                nc.gpsimd.dma_start(out=xv_scr, in_=xv_sb)
                xv_flat = bass.AP(
                    tensor=xv_scr.tensor,
                    offset=xv_scr.offset,
                    ap=[[0, 1], [1, B * HD]],
                )
                nc.gpsimd.dma_start(out=xv1p, in_=xv_flat)

            # ---------------- phase B: transposed scores ----------------
            # new-token chunk: partitions 1.. never written -> -inf
            nc.vector.memset(sT_sb[:, NCH - 1, :], NEG_BIG)

            with tc.tile_pool(name="pqsh", bufs=2, space="PSUM") as pqsh, \
                 tc.tile_pool(name="pqk", bufs=4, space="PSUM") as pqk, \
                 tc.tile_pool(name="pqn", bufs=1, space="PSUM") as pqn:
                # shared prefix: all 64 q rows at once per j-chunk
                for c in range(SH_CH):
                    qs = pqsh.tile([128, R], f32, tag="qksh", name="qs")
                    nc.tensor.matmul(
                        qs,
                        mm(shkT_sb[:, 128 * c : 128 * (c + 1)]),
                        mm(qT_sb),
                        start=True, stop=True,
                    )
                    nc.vector.tensor_copy(sT_sb[:, c, :], qs)

                # per-batch cache scores: kT chunk stationary, q cols moving;
                # all 12 j-chunks of a batch share one PSUM bank -> one evac
                for grp in range(NPAIR // 2):
                    kt = kpool.tile([128, 4 * rsp], DT, tag="kt", name="kt")
                    nc.sync.dma_start(out=kt, in_=kT_d[grp])
                    for j in range(4):
                        b = 4 * grp + j
                        ktb = kt[:, rsp * j : rsp * (j + 1)]
                        rhs = mm(qT_sb[:, NH * b : NH * (b + 1)])
                        qk = pqk.tile([128, BCH * NH], f32, tag="qkb", name="qk")
                        for c in range(BCH):
                            nc.tensor.matmul(
                                qk[:, NH * c : NH * (c + 1)],
                                mm(ktb[:, 128 * c : 128 * (c + 1)]),
                                rhs,
                                start=True, stop=True,
                            )
                        out_ap = sT_sb[
                            :, SH_CH : SH_CH + BCH, NH * b : NH * (b + 1)
                        ]
                        nc.vector.tensor_copy(
                            out_ap, qk.rearrange("p (c n) -> p c n", n=NH)
                        )
                        qn = pqn.tile([1, NH], f32, tag="qn", name="qn")
                        nc.tensor.matmul(
                            qn, mm(xkT_sb[:, b : b + 1]), rhs,
                            start=True, stop=True,
                        )
                        nc.vector.tensor_copy(
                            sT_sb[0:1, NCH - 1, NH * b : NH * (b + 1)], qn
                        )
                    if grp == 1:
                        nc.scalar.activation(
                            out=pT_sb[:, :, : R // 2],
                            in_=sT_sb[:, :, : R // 2],
                            func=AF.Exp, scale=SOFTMAX_SCALE,
                        )

            # prefetch all of v, then the output projection weight, in
            # consumption order (SP issues these after the kT stream)
            vts = []
            for grp in range(NPAIR // 2):
                vt = vpool.tile([128, 4 * rsp], DT, tag="vt", name="vt")
                nc.sync.dma_start(out=vt, in_=v_d[grp])
                vts.append(vt)
            wots = []
            for nn in range(4):
                wot = wopool.tile([128, 2 * 4 * 512], DT, tag="wo", name="wot")
                nc.sync.dma_start(out=wot, in_=wo_d[nn])
                wots.append(wot)

            # ---------------- softmax (no max-sub; |s|<~8) ----------------
            nc.scalar.activation(
                out=pT_sb[:, :, R // 2 :], in_=sT_sb[:, :, R // 2 :],
                func=AF.Exp, scale=SOFTMAX_SCALE,
            )
            def sums_and_shared_pv():
                    # shared-prefix PV for all 64 (b,h) rows, one transpose into
                    # attnT orientation
                    with tc.tile_pool(name="psh", bufs=1, space="PSUM") as psh:
                        pvsh = psh.tile([R, HD], f32)
                        for c in range(SH_CH):
                            nc.tensor.matmul(
                                pvsh,
                                mm(pT_sb[:, c, :]),
                                mm(shv_sb[:, 128 * c : 128 * (c + 1)]),
                                start=(c == 0), stop=(c == SH_CH - 1),
                            )
                        nc.scalar.activation(out=pvsh_sb, in_=pvsh, func=AF.Copy)
                    with tc.tile_pool(name="psht", bufs=1, space="PSUM") as psht:
                        tsh = psht.tile([128, R], DT)
                        nc.tensor.transpose(tsh, pvsh_sb, id_sb)
                        nc.vector.tensor_copy(
                            attnTsh_sb.rearrange("p (h b) -> p b h", b=B),
                            tsh.rearrange("p (b h) -> p b h", h=NH),
                        )

                    with tc.tile_pool(name="ps1", bufs=1, space="PSUM") as ps1:
                        s1 = ps1.tile([1, R], f32)
                        for c in range(NCH):
                            nc.tensor.matmul(
                                s1, mm(ones_sb), mm(pT_sb[:, c, :]),
                                start=(c == 0), stop=(c == NCH - 1),
                            )
                        nc.vector.tensor_copy(sum1_sb, s1)
                    nc.vector.reciprocal(rinv1_sb, sum1_sb)
                    rinv1_hb = const.tile([1, R], DT)
                    nc.vector.tensor_copy(
                        rinv1_hb.rearrange("p (h b) -> p h b", b=B),
                        rinv1_sb.rearrange("p (b h) -> p h b", h=NH),
                    )
                    with tc.tile_pool(name="prb", bufs=1, space="PSUM") as prb:
                        rb_ps = prb.tile([128, R], f32)
                        nc.tensor.matmul(
                            rb_ps, mm(ones1p), mm(rinv1_hb), start=True, stop=True
                        )
                        nc.vector.tensor_copy(rinv_bc, rb_ps)


            # ---------------- PV (batch chunks + new token) ----------------
            with tc.tile_pool(name="ppv", bufs=4, space="PSUM") as ppv, \
                 tc.tile_pool(name="ptrPV", bufs=2, space="PSUM") as ptrPV:

                def pv_transpose(b, pv_sb):
                    tpv = ptrPV.tile([128, NH], DT, tag="tpv", name="tpv")
                    nc.tensor.transpose(tpv, pv_sb, id_sb[:NH, :NH])
                    out_ap = attnT_sb.rearrange(
                        "p (h b) -> p b h", b=B
                    )[:, b, :]
                    nc.vector.tensor_copy(out_ap, tpv)

                pending = None
                for grp in range(NPAIR // 2):
                    vt = vts[grp]
                    for j in range(4):
                        b = 4 * grp + j
                        vb = vt[:, rsp * j : rsp * (j + 1)]
                        pv = ppv.tile([NH, HD], f32, tag="pv", name="pv")
                        for c in range(BCH):
                            nc.tensor.matmul(
                                pv,
                                mm(pT_sb[:, SH_CH + c, NH * b : NH * (b + 1)]),
                                mm(vb[:, 128 * c : 128 * (c + 1)]),
                                start=(c == 0), stop=False,
                            )
                        nc.tensor.matmul(
                            pv,
                            mm(pT_sb[0:1, NCH - 1, NH * b : NH * (b + 1)]),
                            mm(xv1p[:, HD * b : HD * (b + 1)]),
                            start=False, stop=True,
                        )
                        pv_sb = tmp.tile([NH, HD], DT, tag="pvsb", name="pv_sb")
                        nc.scalar.activation(out=pv_sb, in_=pv, func=AF.Copy)
                        if pending is not None:
                            pv_transpose(*pending)
                        pending = (b, pv_sb)
                    if grp == NPAIR // 2 - 2:
                        sums_and_shared_pv()
                pv_transpose(*pending)

            # add shared-prefix part and normalize columns by 1/rowsum
            nc.vector.tensor_add(attnTn_sb, attnT_sb, attnTsh_sb)
            nc.vector.tensor_mul(attnTn_sb, attnTn_sb, rinv_bc)

            # ---------------- phase E: output projection ----------------
            with tc.tile_pool(name="py", bufs=4, space="PSUM") as py:
                for n in range(8):
                    wot = wots[n // 2]
                    off = 4 * 512 * (n % 2)
                    y_ps = py.tile([B, 512], f32, tag="y", name="y_ps")
                    for g in range(4):
                        nc.tensor.matmul(
                            y_ps,
                            mm(attnTn_sb[:, B * g : B * (g + 1)]),
                            mm(wot[:, off + 512 * g : off + 512 * (g + 1)]),
                            start=(g == 0), stop=(g == 3),
                        )
                    nc.vector.tensor_copy(
                        y_sb[:, 512 * n : 512 * (n + 1)], y_ps
                    )
                    if n == 3:
                        nc.sync.dma_start(
                            out=y_d[:, : 4 * 512], in_=y_sb[:, : 4 * 512]
                        )
            nc.sync.dma_start(out=y_d[:, 4 * 512 :], in_=y_sb[:, 4 * 512 :])

    if os.environ.get("KERNEL_SKIP_LEGALIZE") != "1":
        _legalize_multiwait(nc)
    return nc


# ----------------------------------------------------------------------------
# host-side sharding / layout prep
# ----------------------------------------------------------------------------


def _np_dt(dt_name):
    if dt_name == "bfloat16":
        import ml_dtypes

        return ml_dtypes.bfloat16
    return np.float32


def _prep_inputs(inputs, spl, rsp, dt_name):
    nd = _np_dt(dt_name)
    x = np.asarray(inputs["x"], np.float32)            # [16, 1, 4096]
    wq = np.asarray(inputs["wq"], np.float32)
    wk = np.asarray(inputs["wk"], np.float32)
    wv = np.asarray(inputs["wv"], np.float32)
    wo = np.asarray(inputs["wo"], np.float32)
    ck = np.asarray(inputs["cache_k"], np.float32)     # [16, 4096, 8, 128]
    cv = np.asarray(inputs["cache_v"], np.float32)
    shk = np.asarray(inputs["shared_cache_k"], np.float32)  # [1, 512, 8, 128]
    shv = np.asarray(inputs["shared_cache_v"], np.float32)
    cos = np.asarray(inputs["freqs_cos"], np.float32)  # [1, 64]
    sin = np.asarray(inputs["freqs_sin"], np.float32)

    xm = x[:, 0, :]                                    # [16, 4096]
    xT = np.ascontiguousarray(xm.T)                    # [4096, 16]
    xT_p = np.ascontiguousarray(
        xT.reshape(32, 128, B).transpose(1, 0, 2)
    ).reshape(128, 32 * B).astype(nd)

    # rope constants replicated over batch partitions; head-tiled for q
    crep = np.tile(cos.reshape(1, 1, 64), (B, NH, 1)).reshape(B, NH * 64)
    srep = np.tile(sin.reshape(1, 1, 64), (B, NH, 1)).reshape(B, NH * 64)
    rpack = np.ascontiguousarray(
        np.concatenate([crep, srep], axis=1), np.float32
    )

    in_maps = []
    for m in range(N_CORES):
        wqm = wq[:, 512 * m : 512 * (m + 1)]           # [4096, 512]
        wq_p = np.ascontiguousarray(
            wqm.reshape(2, 16, 128, 512).transpose(0, 2, 1, 3)
        ).reshape(2, 128, 16 * 512).astype(nd)
        wkvm = np.concatenate(
            [wk[:, 128 * m : 128 * (m + 1)], wv[:, 128 * m : 128 * (m + 1)]],
            axis=1,
        )                                              # [4096, 256]
        wkv_p = np.ascontiguousarray(
            wkvm.reshape(32, 128, 256).transpose(1, 0, 2)
        ).reshape(128, 32 * 256).astype(nd)
        wom = wo[512 * m : 512 * (m + 1), :]           # [512, 4096]
        wo_p = (
            wom.reshape(4, 128, 8, 512).transpose(2, 1, 0, 3)
        ).reshape(8, 128, 4 * 512)
        wo_p = np.ascontiguousarray(
            wo_p.reshape(4, 2, 128, 4 * 512).transpose(0, 2, 1, 3)
        ).reshape(4, 128, 2 * 4 * 512).astype(nd)

        # kT: [b, hd, j]; 4 batches side by side on the free dim
        ckm = ck[:, :rsp, m, :]                        # [16, rsp, 128]
        kT_p = np.ascontiguousarray(
            ckm.transpose(0, 2, 1).reshape(B // 4, 4, 128, rsp)
            .transpose(0, 2, 1, 3)
        ).reshape(B // 4, 128, 4 * rsp).astype(nd)

        # v: partition-major [b, p, (c d)] with v[b, 128c+p, d] at [p, c, d]
        cvm = cv[:, :rsp, m, :]                        # [16, rsp, 128]
        v_pm = cvm.reshape(B, rsp // 128, 128, 128).transpose(0, 2, 1, 3)
        v_pm = v_pm.reshape(B, 128, rsp)
        v_p = np.ascontiguousarray(
            v_pm.reshape(B // 4, 4, 128, rsp).transpose(0, 2, 1, 3)
        ).reshape(B // 4, 128, 4 * rsp).astype(nd)

        shkT_p = shk[0, :spl, m, :].T
        shv_p = (
            shv[0, :spl, m, :].reshape(spl // 128, 128, 128).transpose(1, 0, 2)
        ).reshape(128, spl)
        cpack = np.concatenate(
            [xT_p.astype(np.float32), shkT_p, shv_p], axis=1
        ).astype(nd)

        in_maps.append(
            {
                "cpack": cpack,
                "wq": wq_p,
                "wkv": wkv_p,
                "wo": wo_p,
                "kT": kT_p,
                "v": v_p,
                "rpack": rpack,
            }
        )
    return in_maps


# ----------------------------------------------------------------------------
# entry point
# ----------------------------------------------------------------------------

_NC_CACHE = {}


def get_nc(spl=512, rsp=1536):
    key = (spl, rsp, STREAM_DTYPE, F32R)
    if key not in _NC_CACHE:
        _patch_tile_drain()
        _install_ntff_hook()
        _NC_CACHE[key] = _build_nc(spl, rsp, STREAM_DTYPE)
    return _NC_CACHE[key]


def prep_inputs(inputs):
    start_pos = int(inputs["start_pos"])
    spl = int(inputs["shared_prefix_length"])
    return _prep_inputs(inputs, spl, start_pos - spl, STREAM_DTYPE)


def kernel(**inputs):
    from concourse.bass_utils import run_bass_kernel_spmd

    start_pos = int(inputs["start_pos"])
    spl = int(inputs["shared_prefix_length"])
    rsp = start_pos - spl
    nc = get_nc(spl, rsp)
    in_maps = _prep_inputs(inputs, spl, rsp, STREAM_DTYPE)
    trace = os.environ.get("KERNEL_TRACE", "0") == "1"
    kwargs = {}
    if trace:
        kwargs = dict(
            trace=True,
            trace_cores=list(range(N_CORES)),
        )
    res = run_bass_kernel_spmd(
        nc, in_maps, core_ids=list(range(N_CORES)), **kwargs
    )
    kernel.last_result = res
    y = np.zeros((B, DIM), np.float64)
    for r in res.results:
        y += r["y"].astype(np.float64)
    return y.reshape(B, 1, DIM).astype(np.float32)



# revision 9
# speedup vs baseline: 1.3835x; 1.3835x over previous
"""Trainium2 Bass kernel for sparse (shared-prefix) GQA decode attention.

Full-input contract: kernel(**inputs) takes the unsharded tensors from
setup_inputs() and returns the full [16, 1, 4096] float32 output.

Sharding: tensor-parallel over heads across 8 NeuronCores. Core m owns
query heads 4m..4m+3 and kv head m (GQA group m): wq columns
[512m, 512m+512), wk/wv columns [128m, 128m+128), wo rows [512m, 512m+512),
and head m of the kv caches. Each core computes a partial y_m = attn_m @
wo_m; the host sums the 8 partials (the "all-reduce").

Key design points vs the earlier version of this kernel:
  * kv cache and wk/wv stream in fp8 e3m4 (4-bit mantissa); wq/wo stay
    bf16. Mixed-dtype matmuls (fp8 stationary x bf16 moving) are legal on
    TRN2, so q and the probabilities stay bf16. ~15.6MB/core HBM traffic.
  * RoPE is folded into wq/wk on the host (seqlen=1: one rotation matrix,
    a host-side weight reparameterization), so the device needs no rope
    and no transposes: projections run weight-stationary and produce
    qT/xkT/xvT directly in [d, batch] orientation.
  * PV runs v-stationary (lhsT = v chunk [j,128d], moving = probs
    [j,4h]), so attention output lands as attnT [128d, (b,h)] with no
    per-batch PE transposes.
  * Everything is SBUF-resident; input DMAs are issued up front in
    consumption order as ~1MB pieces on the single SP HWDGE FIFO, so the
    HBM stream stays saturated start to finish.
  * PE instruction order is arranged to match data arrival (single
    in-order queue), with a dummy-matmul warmup to beat the HAM clock
    gate and a dummy Exp to preload the activation table.

Problem constants (hardcoded per the harness contract): bsz=16, seqlen=1,
dim=4096, n_heads=32, n_kv=8, hd=128, start_pos=2048,
shared_prefix_length=512 -> rsp=1536, L=2049.
"""

import math
import os
import sys
import types

import numpy as np

# ----------------------------------------------------------------------------
# environment patches (self-contained; no /root/problem reads)
# ----------------------------------------------------------------------------


def _patch_tile_drain():
    """The stock TileContext._drain_and_barrier puts one sem-wait per live
    semaphore on a single Drain instruction; the walrus build in this image
    only accepts a single sync wait per instruction ("Too many sync wait
    commands"). Re-emit the waits as individual EventSemaphore instructions
    on the same sequencer instead."""
    import concourse.tile as tile
    from concourse.vector_clock import ScopedClock

    if getattr(tile.TileContext, "_drain_patched", False):
        return

    def _drain_and_barrier(self, tick_clock, wait_clock):
        nc = self.nc
        drain_inst = nc.sync.drain()
        wait_clock.add_sem_waits(
            drain_inst.ins, ScopedClock({None: tick_clock.global_clock})
        )
        waits = list(drain_inst.ins.sync_info.on_wait)
        if len(waits) > 1:
            by_name = {h.name: h for h in self.sems.allocated().values()}
            try:
                drain_inst.ins.sync_info = None
            except Exception:
                pass
            for w in waits:
                h = by_name.get(w.ant_name)
                assert h is not None, f"no handle for sem {w.ant_name}"
                nc.sync.wait_ge(h, w.wait_value)

        assert self.sems is not None
        popped = nc._tile_sem_poison_stack.pop()
        assert popped is self._sem_poison
        nums = [h.num for h in self.sems.allocated().values()]
        nc._state.prepend_free_semaphores(nums)
        for ps in nc._tile_sem_poison_stack:
            ps.update(nums)

    tile.TileContext._drain_and_barrier = _drain_and_barrier
    tile.TileContext._drain_patched = True


def _install_ntff_hook():
    """Optional: register the axon NTFF profile hook (missing from the
    trimmed antenv package) so trace=True works for profiling, and stub the
    S3 artifact upload (zero-egress container)."""
    try:
        if "antenv.axon_hooks" not in sys.modules:
            mod = types.ModuleType("antenv.axon_hooks")
            mod._hook = None
            mod.set_axon_ntff_profile_hook = lambda h: setattr(mod, "_hook", h)
            mod.get_axon_ntff_profile_hook = lambda: mod._hook
            sys.modules["antenv.axon_hooks"] = mod
            import antenv

            antenv.axon_hooks = mod
            from trn_agent_boot.trn_boot import _ntff_profile_via_ctypes

            mod.set_axon_ntff_profile_hook(
                _ntff_profile_via_ctypes("/opt/axon/libaxon_pjrt.so")
            )
        import concourse.bass_utils as bu

        bu.upload_artifacts = lambda tmpdir: tmpdir
    except Exception:
        pass


def _legalize_multiwait(nc, max_waits=1):
    """This walrus build accepts at most one sync wait per instruction.
    Hoist excess waits into standalone single-wait EventSemaphore
    instructions inserted immediately before, on the same engine."""
    import bass_rust

    uid = 0
    for f in nc.m.functions:
        for bb in f.blocks:
            insts = list(bb.instructions)
            out = []
            changed = False
            for ins in insts:
                si = ins.sync_info
                if si is not None:
                    waits = list(si.on_wait)
                    if len(waits) > max_waits:
                        for w in waits[:-max_waits]:
                            ev = bass_rust.InstEventSemaphore(
                                name=f"{ins.name}_xw{uid}"
                            )
                            uid += 1
                            ev.engine = ins.engine
                            ev.sync_info = bass_rust.SyncInfo(
                                on_wait=[w], on_update=[]
                            )
                            out.append(ev)
                        ins.sync_info = bass_rust.SyncInfo(
                            on_wait=waits[-max_waits:],
                            on_update=list(si.on_update),
                        )
                        changed = True
                out.append(ins)
            if changed:
                bb.instructions = out


# ----------------------------------------------------------------------------
# constants
# ----------------------------------------------------------------------------

N_CORES = 8
B = 16            # batch
DIM = 4096
N_HEADS = 32
N_KV = 8
HD = 128
NH = N_HEADS // N_CORES      # 4 local q heads
R = B * NH                   # 64 cols, r = 4*b + h
SOFTMAX_SCALE = 1.0 / math.sqrt(HD)
WS = 32.0                    # host pre-scale on wk/wv so e3m4 normals cover them

# stream dtypes (env-overridable for A/B tests)
CACHE_DT = os.environ.get("KERNEL_CACHE_DT", "float8e3")
WKV_DT = os.environ.get("KERNEL_WKV_DT", "float8e3")
WQ_DT = os.environ.get("KERNEL_WQ_DT", "bfloat16")
WO_DT = os.environ.get("KERNEL_WO_DT", "bfloat16")

WARMUP_MMS = 36


# ----------------------------------------------------------------------------
# device kernel
# ----------------------------------------------------------------------------


def _build_nc(spl, rsp):
    import concourse.bass as bass
    import concourse.tile as tile
    from concourse import mybir
    from concourse.mybir import ActivationFunctionType as AF

    BF = mybir.dt.bfloat16
    F16 = mybir.dt.float16
    f32 = mybir.dt.float32
    CDT = getattr(mybir.dt, CACHE_DT)
    KVDT = getattr(mybir.dt, WKV_DT)
    QDT = getattr(mybir.dt, WQ_DT)
    ODT = getattr(mybir.dt, WO_DT)
    csz = 1 if CACHE_DT == "float8e3" else 2
    kvsz = 1 if WKV_DT == "float8e3" else 2

    assert spl % 128 == 0 and rsp % (128 * 4) == 0
    SH_CH = spl // 128           # shared j-chunks (4)
    BCH = rsp // 128             # per-batch cache j-chunks (12)
    NCH = SH_CH + BCH + 1        # total chunks incl. new-token chunk (17)
    NPC = 4                      # batches per kT/v DMA piece
    NPIECE = B // NPC            # 4 pieces

    nc = bass.Bass(
        "TRN2", target_bir_lowering=False, debug=False, num_devices=N_CORES
    )

    cpack_d = nc.dram_tensor("cpack", [128, 32 * B + 2 * spl], BF,
                             kind="ExternalInput").ap()
    wq_d = nc.dram_tensor("wq", [4, 128, 8 * NH * HD], QDT,
                          kind="ExternalInput").ap()
    wkv_d = nc.dram_tensor("wkv", [128, 32 * 2 * HD], KVDT,
                           kind="ExternalInput").ap()
    kT_d = nc.dram_tensor("kT", [NPIECE, 128, NPC * rsp], CDT,
                          kind="ExternalInput").ap()
    v_d = nc.dram_tensor("v", [NPIECE, 128, NPC * rsp], CDT,
                         kind="ExternalInput").ap()
    wo_d = nc.dram_tensor("wo", [NH, 128, DIM], ODT,
                          kind="ExternalInput").ap()
    y_d = nc.dram_tensor("y", [B, DIM], f32, kind="ExternalOutput").ap()

    with tile.TileContext(nc) as tc:
        with tc.tile_pool(name="const", bufs=1) as const:
            # ---------------- resident SBUF tiles ----------------
            ones_sb = const.tile([128, 1], BF, tag="ones")
            ones1p = const.tile([1, 128], BF, tag="ones1p")
            g64 = const.tile([128, R], BF, tag="g64")
            scr1 = const.tile([1, 4], BF, tag="scr1")
            cpack_sb = const.tile([128, 32 * B + 2 * spl], BF, tag="cpack")
            wq_sb = const.tile([128, 32 * NH * HD], QDT, tag="wq")
            wkv_sb = const.tile([128, 32 * 2 * HD], KVDT, tag="wkv")
            kT_sb = const.tile([128, B * rsp], CDT, tag="kT")
            v_sb = const.tile([128, B * rsp], CDT, tag="v")
            wo_sb = const.tile([128, NH * DIM], ODT, tag="wo")
            qT_sb = const.tile([128, R], BF, tag="qT")
            xkT_sb = const.tile([128, B], BF, tag="xkT")
            xvT_sb = const.tile([128, B], BF, tag="xvT")
            xk_bc = const.tile([128, R], BF, tag="xk_bc")
            xv_bc = const.tile([128, R], BF, tag="xv_bc")
            prod_sb = const.tile([128, R], F16, tag="prod")
            pT = const.tile([128, NCH, R], BF, tag="pT")
            sum1 = const.tile([1, R], f32, tag="sum1")
            rinv1 = const.tile([1, R], BF, tag="rinv1")
            rbc_sb = const.tile([128, 2, R], BF, tag="rbc")  # [pnew_bc|rinv_bc]
            attnT = const.tile([128, R], BF, tag="attnT")    # cols 4b+h
            attnT2 = const.tile([128, R], BF, tag="attnT2")  # cols 16h+b
            y_sb = const.tile([B, DIM], f32, tag="y")

            xT = cpack_sb[:, : 32 * B]
            shkT = cpack_sb[:, 32 * B : 32 * B + spl]
            shv = cpack_sb[:, 32 * B + spl :]

            # ---------------- constants + input DMA stream ----------------
            nc.vector.memset(ones_sb, 1.0)
            nc.vector.memset(ones1p, 1.0)
            nc.vector.memset(g64, 0.0)
            nc.vector.memset(pT[:, NCH - 1, :], 0.0)

            # all input DMAs up front, on the SP HWDGE FIFO, in
            # consumption order: x/shared, wq, kT, wkv, v, wo
            nc.sync.dma_start(out=cpack_sb, in_=cpack_d)
            WQP = 8 * NH * HD
            for g in range(4):
                nc.sync.dma_start(
                    out=wq_sb[:, WQP * g : WQP * (g + 1)], in_=wq_d[g]
                )
            for g in range(NPIECE):
                nc.sync.dma_start(
                    out=kT_sb[:, NPC * rsp * g : NPC * rsp * (g + 1)],
                    in_=kT_d[g],
                )
            nc.sync.dma_start(out=wkv_sb, in_=wkv_d)
            for g in range(NPIECE):
                nc.sync.dma_start(
                    out=v_sb[:, NPC * rsp * g : NPC * rsp * (g + 1)],
                    in_=v_d[g],
                )
            for h in range(NH):
                nc.sync.dma_start(
                    out=wo_sb[:, DIM * h : DIM * (h + 1)], in_=wo_d[h]
                )

            # preload the ACT Exp table during the DMA dead time
            nc.scalar.activation(out=scr1, in_=g64[0:1, 0:4], func=AF.Exp)

            # ---------------- PE warmup (HAM clock gate) ----------------
            with tc.tile_pool(name="pwarm", bufs=1, space="PSUM") as pw:
                wps = pw.tile([1, R], f32, tag="wps")
                for i in range(WARMUP_MMS):
                    nc.tensor.matmul(wps, ones_sb, g64, start=True, stop=True)

            # ---------------- long-lived PSUM pools ----------------
            with tc.tile_pool(name="ppv", bufs=1, space="PSUM") as ppvp, \
                 tc.tile_pool(name="psum1", bufs=1, space="PSUM") as psump, \
                 tc.tile_pool(name="pnew", bufs=1, space="PSUM") as pnewp:
                ps_pv = ppvp.tile([128, R], f32, tag="pv")
                ps_sum = psump.tile([1, R], f32, tag="sum")
                ps_snew = pnewp.tile([1, R], f32, tag="snew")
                ps_bc = pnewp.tile([128, 2, R], f32, tag="bc")

                # ---------------- q projection (weight-stationary) --------
                # NB: start=True resets has_written for the WHOLE psum bank,
                # so every concurrently-accumulating group needs its own bank
                # (PSUM pool slots are bank-granular).
                with tc.tile_pool(name="psq", bufs=1, space="PSUM") as psqp:
                    psq = [psqp.tile([128, B], f32, tag=f"psq{h}",
                                     name=f"psq{h}") for h in range(NH)]
                    for kc in range(32):
                        rx = xT[:, B * kc : B * (kc + 1)]
                        for h in range(NH):
                            nc.tensor.matmul(
                                psq[h],
                                wq_sb[:, (kc * NH + h) * HD : (kc * NH + h + 1) * HD],
                                rx,
                                start=(kc == 0),
                                stop=(kc == 31),
                            )
                    qTv = qT_sb.rearrange("p (b h) -> p b h", h=NH)
                    for h in range(NH):
                        nc.vector.tensor_copy(qTv[:, :, h], psq[h])

                # ---------------- shared-prefix scores + PV ----------------
                with tc.tile_pool(name="psh", bufs=1, space="PSUM") as pshp:
                    ps_sh = pshp.tile([128, SH_CH, R], f32, tag="sh")
                    for c in range(SH_CH):
                        nc.tensor.matmul(
                            ps_sh[:, c, :],
                            shkT[:, 128 * c : 128 * (c + 1)],
                            qT_sb,
                            start=True, stop=True,
                        )
                    nc.scalar.activation(
                        out=pT[:, 0:SH_CH, :], in_=ps_sh,
                        func=AF.Exp, scale=SOFTMAX_SCALE,
                    )
                # shared PV opens the big accumulation into ps_pv
                for c in range(SH_CH):
                    nc.tensor.matmul(
                        ps_pv,
                        shv[:, 128 * c : 128 * (c + 1)],
                        pT[:, c, :],
                        start=(c == 0), stop=False,
                        skip_group_check=True,
                    )
                # shared part of the softmax denominators
                for c in range(SH_CH):
                    nc.tensor.matmul(
                        ps_sum, ones_sb, pT[:, c, :],
                        start=(c == 0), stop=False,
                        skip_group_check=True,
                    )

                # ---------------- per-batch cache scores ----------------
                pTc = pT[:, SH_CH : SH_CH + BCH, :].rearrange(
                    "p c (g r2) -> p c g r2", r2=2 * NH
                )
                with tc.tile_pool(name="pqk", bufs=3, space="PSUM") as pqkp:
                    for grp in range(B // 2):       # 2 batches per psum tile
                        qk = pqkp.tile([128, BCH, 2 * NH], f32, tag="qk",
                                       name=f"qk{grp}")
                        for b2 in range(2):
                            b = 2 * grp + b2
                            rq = qT_sb[:, NH * b : NH * (b + 1)]
                            for c in range(BCH):
                                nc.tensor.matmul(
                                    qk[:, c, NH * b2 : NH * (b2 + 1)],
                                    kT_sb[:, rsp * b + 128 * c : rsp * b + 128 * (c + 1)],
                                    rq,
                                    start=True, stop=True,
                                )
                        nc.scalar.activation(
                            out=pTc[:, :, grp, :], in_=qk,
                            func=AF.Exp, scale=SOFTMAX_SCALE,
                        )

                # ---------------- k/v projection ----------------
                with tc.tile_pool(name="pskv", bufs=1, space="PSUM") as pskvp:
                    pskv = [pskvp.tile([128, B], f32, tag=f"pskv{u}",
                                       name=f"pskv{u}") for u in range(2)]
                    for kc in range(32):
                        rx = xT[:, B * kc : B * (kc + 1)]
                        for u in range(2):
                            nc.tensor.matmul(
                                pskv[u],
                                wkv_sb[:, (kc * 2 + u) * HD : (kc * 2 + u + 1) * HD],
                                rx,
                                start=(kc == 0),
                                stop=(kc == 31),
                            )
                    nc.scalar.activation(out=xkT_sb, in_=pskv[0],
                                         func=AF.Copy, scale=1.0 / WS)
                    nc.scalar.activation(out=xvT_sb, in_=pskv[1],
                                         func=AF.Copy, scale=1.0 / WS)

                # cache-chunk part of the denominators
                for c in range(BCH):
                    nc.tensor.matmul(
                        ps_sum, ones_sb, pT[:, SH_CH + c, :],
                        start=False, stop=False,
                        skip_group_check=True,
                    )

                # new-token score via DVE product + ones-matmul reduction
                xkv_ = xk_bc.rearrange("p (b h) -> p b h", h=NH)
                xvv_ = xv_bc.rearrange("p (b h) -> p b h", h=NH)
                for h in range(NH):
                    nc.vector.tensor_copy(xkv_[:, :, h], xkT_sb)
                    nc.vector.tensor_copy(xvv_[:, :, h], xvT_sb)
                nc.vector.tensor_mul(prod_sb, qT_sb, xk_bc)

                def pv_piece(g):
                    for j in range(NPC):
                        b = NPC * g + j
                        for c in range(BCH):
                            nc.tensor.matmul(
                                ps_pv[:, NH * b : NH * (b + 1)],
                                v_sb[:, rsp * b + 128 * c : rsp * b + 128 * (c + 1)],
                                pT[:, SH_CH + c, NH * b : NH * (b + 1)],
                                start=False, stop=(c == BCH - 1),
                                skip_group_check=True,
                            )

                # ---------------- PV + new-token/denominator chain --------
                pv_piece(0)

                nc.tensor.matmul(ps_snew, ones_sb, prod_sb,
                                 start=True, stop=True)
                nc.scalar.activation(
                    out=pT[0:1, NCH - 1, :], in_=ps_snew,
                    func=AF.Exp, scale=SOFTMAX_SCALE,
                )

                pv_piece(1)
                pv_piece(2)

                # finish denominators (new-token chunk; rows 1.. are zero)
                nc.tensor.matmul(
                    ps_sum, ones_sb, pT[:, NCH - 1, :],
                    start=False, stop=True, skip_group_check=True,
                )
                nc.vector.tensor_copy(sum1, ps_sum)
                with nc.allow_low_precision(reason="1/rowsum to bf16 is fine"):
                    nc.vector.reciprocal(rinv1, sum1)
                # broadcast p_new and 1/rowsum down the partitions
                nc.tensor.matmul(ps_bc[:, 0, :], ones1p, pT[0:1, NCH - 1, :],
                                 start=True, stop=True)
                nc.tensor.matmul(ps_bc[:, 1, :], ones1p, rinv1,
                                 start=True, stop=True)
                nc.vector.tensor_copy(rbc_sb, ps_bc)

                pv_piece(3)

                # evac attention output per 4-batch column range, add the
                # new-token term, normalize, reorder cols (4b+h)->(16h+b)
                a2v = attnT2.rearrange("p (h b) -> p b h", b=B)
                pnew_bc = rbc_sb[:, 0, :]
                rinv_bc = rbc_sb[:, 1, :]
                rbv_all = rinv_bc.rearrange("p (b h) -> p b h", h=NH)
                inv_all = attnT.rearrange("p (b h) -> p b h", h=NH)
                for g in range(NPIECE):
                    S = slice(NH * NPC * g, NH * NPC * (g + 1))
                    bs = slice(NPC * g, NPC * (g + 1))
                    nc.vector.tensor_copy(attnT[:, S], ps_pv[:, S])
                    nc.vector.tensor_mul(xv_bc[:, S], xv_bc[:, S],
                                         pnew_bc[:, S])
                    nc.vector.tensor_add(attnT[:, S], attnT[:, S],
                                         xv_bc[:, S])
                    nc.vector.tensor_mul(a2v[:, bs, :], inv_all[:, bs, :],
                                         rbv_all[:, bs, :])

            # ---------------- output projection ----------------
            with tc.tile_pool(name="py", bufs=1, space="PSUM") as pyp:
                ys = [pyp.tile([B, 512], f32, tag=f"y{n}", name=f"ys{n}")
                      for n in range(8)]
                for h in range(NH):
                    lq = attnT2[:, B * h : B * (h + 1)]
                    for n in range(8):
                        nc.tensor.matmul(
                            ys[n],
                            lq,
                            wo_sb[:, DIM * h + 512 * n : DIM * h + 512 * (n + 1)],
                            start=(h == 0),
                            stop=(h == NH - 1),
                        )
                        if h == NH - 1:
                            nc.vector.tensor_copy(
                                y_sb[:, 512 * n : 512 * (n + 1)], ys[n]
                            )
                            if n == 3:
                                nc.sync.dma_start(
                                    out=y_d[:, : 4 * 512],
                                    in_=y_sb[:, : 4 * 512],
                                )
            nc.sync.dma_start(out=y_d[:, 4 * 512 :], in_=y_sb[:, 4 * 512 :])

            if os.environ.get("KERNEL_DEBUG") == "1":
                def dbg(name, ap):
                    d = nc.dram_tensor(
                        f"dbg_{name}", list(ap.shape), ap.dtype,
                        kind="ExternalOutput",
                    ).ap()
                    nc.sync.dma_start(out=d, in_=ap)
                dbg("qT", qT_sb)
                dbg("xkT", xkT_sb)
                dbg("xvT", xvT_sb)
                dbg("pT", pT)
                dbg("sum1", sum1)
                dbg("rbc", rbc_sb)
                dbg("attnT", attnT)
                dbg("attnT2", attnT2)
                dbg("prod", prod_sb)

    if os.environ.get("KERNEL_SKIP_LEGALIZE") != "1":
        _legalize_multiwait(nc)
    return nc


# ----------------------------------------------------------------------------
# host-side sharding / layout prep
# ----------------------------------------------------------------------------


def _np_dt(name):
    import ml_dtypes

    return {
        "bfloat16": ml_dtypes.bfloat16,
        "float8e3": ml_dtypes.float8_e3m4,
        "float16": np.float16,
        "float32": np.float32,
    }[name]


def _prep_inputs(inputs, spl, rsp):
    x = np.asarray(inputs["x"], np.float32)            # [16, 1, 4096]
    wq = np.asarray(inputs["wq"], np.float32)
    wk = np.asarray(inputs["wk"], np.float32)
    wv = np.asarray(inputs["wv"], np.float32)
    wo = np.asarray(inputs["wo"], np.float32)
    ck = np.asarray(inputs["cache_k"], np.float32)     # [16, 4096, 8, 128]
    cv = np.asarray(inputs["cache_v"], np.float32)
    shk = np.asarray(inputs["shared_cache_k"], np.float32)  # [1, 512, 8, 128]
    shv = np.asarray(inputs["shared_cache_v"], np.float32)
    cos = np.asarray(inputs["freqs_cos"], np.float32)[0]    # [64]
    sin = np.asarray(inputs["freqs_sin"], np.float32)[0]

    bdt = _np_dt("bfloat16")
    cdt = _np_dt(CACHE_DT)
    kvdt = _np_dt(WKV_DT)
    qdt = _np_dt(WQ_DT)
    odt = _np_dt(WO_DT)

    def fold_rope(w):
        # seqlen=1 decode: rope is one fixed pairwise rotation; fold it
        # into the projection columns (a host-side reparameterization)
        W = w.reshape(w.shape[0], -1, 64, 2)
        we, wo_ = W[..., 0], W[..., 1]
        return np.stack(
            [we * cos - wo_ * sin, we * sin + wo_ * cos], -1
        ).reshape(w.shape)

    wq_r = fold_rope(wq)
    wk_r = fold_rope(wk) * WS
    wv_s = wv * WS

    xm = x[:, 0, :]                                    # [16, 4096]
    xT_p = np.ascontiguousarray(
        xm.T.reshape(32, 128, B).transpose(1, 0, 2)
    ).reshape(128, 32 * B)

    BCH = rsp // 128
    in_maps = []
    for m in range(N_CORES):
        # wq': pieces of 8 k-chunks; col ((kc%8)*4+h)*128+d
        wqm = wq_r[:, 512 * m : 512 * (m + 1)]         # [4096, 512]
        wq_p = np.ascontiguousarray(
            wqm.reshape(4, 8, 128, NH * HD).transpose(0, 2, 1, 3)
        ).reshape(4, 128, 8 * NH * HD).astype(qdt)

        # wkv': col (kc*2+u)*128+d
        wkvm = np.concatenate(
            [wk_r[:, 128 * m : 128 * (m + 1)], wv_s[:, 128 * m : 128 * (m + 1)]],
            axis=1,
        )                                              # [4096, 256]
        wkv_p = np.ascontiguousarray(
            wkvm.reshape(32, 128, 256).transpose(1, 0, 2)
        ).reshape(128, 32 * 256).astype(kvdt)

        # kT: [hd, j] per batch; 4 batches per piece
        ckm = ck[:, :rsp, m, :]                        # [16, rsp, 128]
        kT_p = np.ascontiguousarray(
            ckm.transpose(0, 2, 1).reshape(4, 4, 128, rsp).transpose(0, 2, 1, 3)
        ).reshape(4, 128, 4 * rsp).astype(cdt)

        # v: [j%128, (b,c,d)]; v[b, 128c+p, d] at [p, (b*BCH+c)*128+d]
        cvm = cv[:, :rsp, m, :]                        # [16, rsp, 128]
        v_p = np.ascontiguousarray(
            cvm.reshape(4, 4, BCH, 128, 128).transpose(0, 3, 1, 2, 4)
        ).reshape(4, 128, 4 * rsp).astype(cdt)

        # wo rows for this core: [h, d, n]
        wom = wo[512 * m : 512 * (m + 1), :]           # [512, 4096]
        wo_p = np.ascontiguousarray(
            wom.reshape(NH, 128, DIM)
        ).astype(odt)

        shkT_p = shk[0, :spl, m, :].T                  # [128, spl]
        shv_p = (
            shv[0, :spl, m, :].reshape(spl // 128, 128, 128).transpose(1, 0, 2)
        ).reshape(128, spl)
        cpack = np.concatenate([xT_p, shkT_p, shv_p], axis=1).astype(bdt)

        in_maps.append(
            {
                "cpack": cpack,
                "wq": wq_p,
                "wkv": wkv_p,
                "kT": kT_p,
                "v": v_p,
                "wo": wo_p,
            }
        )
    return in_maps


# ----------------------------------------------------------------------------
# entry point
# ----------------------------------------------------------------------------

_NC_CACHE = {}


def get_nc(spl=512, rsp=1536):
    key = (spl, rsp, CACHE_DT, WKV_DT, WQ_DT, WO_DT)
    if key not in _NC_CACHE:
        _patch_tile_drain()
        _install_ntff_hook()
        _NC_CACHE[key] = _build_nc(spl, rsp)
    return _NC_CACHE[key]


def prep_inputs(inputs):
    start_pos = int(inputs["start_pos"])
    spl = int(inputs["shared_prefix_length"])
    return _prep_inputs(inputs, spl, start_pos - spl)


def kernel(**inputs):
    from concourse.bass_utils import run_bass_kernel_spmd

    start_pos = int(inputs["start_pos"])
    spl = int(inputs["shared_prefix_length"])
    rsp = start_pos - spl
    nc = get_nc(spl, rsp)
    in_maps = _prep_inputs(inputs, spl, rsp)
    trace = os.environ.get("KERNEL_TRACE", "0") == "1"
    kwargs = {}
    if trace:
        kwargs = dict(
            trace=True,
            trace_cores=list(range(N_CORES)),
        )
    res = run_bass_kernel_spmd(
        nc, in_maps, core_ids=list(range(N_CORES)), **kwargs
    )
    kernel.last_result = res
    y = np.zeros((B, DIM), np.float64)
    for r in res.results:
        y += r["y"].astype(np.float64)
    return y.reshape(B, 1, DIM).astype(np.float32)
